# revision 28
# baseline (speedup 1.0000x reference)
"""CromLinear (VQ-codebook linear) Trainium2 kernel.

Math: reference computes
    quantized = codebook[indices]                       # [n_blocks, 64]
    w_ste     = continuous_weight + stopgrad(quantized - continuous_weight)
              = quantized                               (exact in fp32 forward)
    W         = w_ste.reshape(4096, 4096)
    out       = x @ W + bias
so continuous_weight cancels out of the forward value; the kernel only needs
the codebook gather + GEMM + bias.

Sharding: column-parallel over 8 cores.  Core c owns out columns
[512c, 512(c+1)).  W's rows are contiguous runs of 64-wide blocks:
W[r, 64c+j] = codebook[idx2d[r, c], j] with idx2d = indices.reshape(4096, 64),
so core c needs idx2d[:, 8c:8c+8].  x is replicated (each output column needs
all of x); bias is split.

Per-core device kernel (v3, see build_nc_v3; build_nc/_v2 are kept as
reference implementations of earlier iterations):
  - W k-tiles are expanded on the PE from a one-hot of the block index
    (is_equal against a per-partition iota on DVE) times the SBUF-resident
    bf16 codebook, then cast PSUM->SBUF bf16 on the otherwise-idle Scalar
    engine.  dma_gather (v1) and per-k-tile x streaming (v2) both lose to
    this: SWDGE descriptor generation costs ~8 ns/block, and every
    128-partition dma_start costs ~630 ns of HWDGE descriptor generation
    (DIRECT2D) on the issuing engine, so DMA count is minimized.
  - x is bf16 and fully SBUF-resident (8 MB), loaded as 8 1-MB group DMAs of
    128 8-KB descriptors; partition p of group g holds k-rows 512g+4p+a, and
    the W build emits the same permuted k-order so the contraction matches.
  - out rows run in two m-groups of 4: pass A builds W tiles (PSUM banks 6-7
    double-buffered) and accumulates m=0..3 (banks 0-3); pass B reuses the
    resident x/W for m=4..7 (banks 4-7) with zero DMA and zero per-tile
    semaphores, m-major so each bias-add/store overlaps the next m's MMs.
  - epilogue: DVE adds bias (psum + bias -> SBUF), sync engine stores out.
"""

import functools

import numpy as np

import concourse.bacc as bacc
import concourse.mybir as mybir
from concourse.bass_utils import run_bass_kernel_spmd
from concourse.library_config import mlp

# Problem shape (hardcoded per the task contract).
M = 1024          # x rows (2*512)
K = 4096          # in_features
N = 4096          # out_features
NCODES = 256
CDIM = 64         # codebook row length (= 256 bytes in f32)
NCORES = 8
NC_COLS = N // NCORES          # 512 out columns per core
NC_CBLK = NC_COLS // CDIM      # 8 column-blocks per core
KT = K // 128                  # 32 k-tiles
MT = M // 128                  # 8 m-tiles
XB = 4                         # x-tile double-buffer depth
WB = 4                         # w-tile double-buffer depth
F32R = mybir.dt.float32r


@functools.lru_cache(maxsize=4)
def build_nc(kt=KT, use_gather=True, use_mm=True, single_packet=True, nqueues=1):
    nc = bacc.Bacc("TRN2", target_bir_lowering=False, debug=False)

    # x / codebook flow through the kernel as float32r (TF32: fp32 with the
    # mantissa rounded to 11 bits, low 12 bits zero — full-rate PE matmul).
    # The host pre-rounds the values, so DMA just moves already-f32r data.
    xt = nc.dram_tensor("xt", [K, M], F32R, kind="ExternalInput")
    cb = nc.dram_tensor("cb", [NCODES, CDIM], F32R, kind="ExternalInput")
    idx = nc.dram_tensor("idx", [128, KT * 64], mybir.dt.int16, kind="ExternalInput")
    bias = nc.dram_tensor("bias", [128, NC_COLS], mybir.dt.float32, kind="ExternalInput")
    out = nc.dram_tensor("out", [M, NC_COLS], mybir.dt.float32, kind="ExternalOutput")

    from contextlib import ExitStack

    with (
        nc.sbuf_tensor("ibuf", [128, KT * 64], mybir.dt.int16) as ibuf,
        nc.sbuf_tensor("btile", [128, NC_COLS], mybir.dt.float32) as btile,
        ExitStack() as stack,
    ):
        xbuf = [
            stack.enter_context(nc.sbuf_tensor(f"xbuf{i}", [128, M], F32R))
            for i in range(XB)
        ]
        wbuf = [
            stack.enter_context(
                nc.sbuf_tensor(f"wbuf{i}", [128, NC_CBLK, CDIM], F32R)
            )
            for i in range(WB)
        ]
        obuf = [
            stack.enter_context(
                nc.sbuf_tensor(f"obuf{m}", [128, NC_COLS], mybir.dt.float32)
            )
            for m in range(MT)
        ]
        psum = [
            stack.enter_context(
                nc.psum_tensor(f"ps{m}", [128, NC_COLS], mybir.dt.float32)
            )
            for m in range(MT)
        ]
        sxs = [stack.enter_context(nc.semaphore(f"sx{i}")) for i in range(XB)]
        sws = [stack.enter_context(nc.semaphore(f"sw{i}")) for i in range(WB)]
        si = stack.enter_context(nc.semaphore("si"))
        sb = stack.enter_context(nc.semaphore("sb"))
        sm = stack.enter_context(nc.semaphore("sm"))
        sv = stack.enter_context(nc.semaphore("sv"))
        so = stack.enter_context(nc.semaphore("so"))

        with nc.Block() as block:

            @block.sync
            def _(sync):
                sync.dma_start(btile[:], bias[:]).then_inc(sb, 16)
                for t in range(kt):
                    if t >= XB:
                        # slot t%XB last used by k-tile t-XB; wait for PE
                        sync.wait_ge(sm, t - XB + 1)
                    sync.dma_start(
                        xbuf[t % XB][:], xt[128 * t : 128 * (t + 1), :]
                    ).then_inc(sxs[t % XB], 16)
                for m in range(MT):
                    sync.wait_ge(sv, m + 1)
                    sync.dma_start(
                        out[128 * m : 128 * (m + 1), :], obuf[m][:]
                    ).then_inc(so, 16)
                sync.wait_ge(so, 16 * MT)

            @block.gpsimd
            def _(gpsimd):
                gpsimd.load_library(mlp)
                gpsimd.dma_start(ibuf[:], idx[:]).then_inc(si, 16)
                gpsimd.wait_ge(si, 16)
                for t in range(kt if use_gather else 0):
                    if t >= WB:
                        gpsimd.wait_ge(sm, t - WB + 1)
                    gpsimd.dma_gather(
                        wbuf[t % WB][:],
                        cb[:],
                        ibuf[:, 64 * t : 64 * (t + 1)],
                        128 * NC_CBLK,
                        128 * NC_CBLK,
                        CDIM,
                        single_packet=single_packet,
                        queue_num=t % nqueues,
                    ).then_inc(sws[t % WB], 16)

            @block.tensor
            def _(tensor):
                for t in range(kt if use_mm else 0):
                    tensor.wait_ge(sxs[t % XB], 16 * (t // XB + 1))
                    if use_gather:
                        tensor.wait_ge(sws[t % WB], 16 * (t // WB + 1))
                    rhs = wbuf[t % WB].ap().rearrange("p a b -> p (a b)")
                    for m in range(MT):
                        ins = tensor.matmul(
                            psum[m][:],
                            xbuf[t % XB][:, 128 * m : 128 * (m + 1)],
                            rhs,
                            start=(t == 0),
                            stop=(t == kt - 1),
                        )
                    ins.then_inc(sm, 1)

            @block.vector
            def _(vector):
                vector.wait_ge(sb, 16)
                if use_mm:
                    vector.wait_ge(sm, kt)
                else:
                    # fake PE progress so sync/gpsimd slot-reuse waits pass
                    pass
                for m in range(MT):
                    vector.tensor_add(obuf[m][:], psum[m][:], btile[:]).then_inc(sv, 1)

    nc.compile()
    return nc


def _round_f32r(a):
    """Round fp32 -> float32r (TF32): round-to-nearest-even at mantissa bit 11,
    low 12 bits zeroed.  Matches the hardware's fp32_to_fp32r layout."""
    u = np.ascontiguousarray(a, dtype=np.float32).view(np.uint32)
    u = (u + 0x7FF + ((u >> 12) & 1)) & np.uint32(0xFFFFF000)
    return u.astype(np.uint32).view(np.float32)


def _prep_inputs(x, codebook, bias, indices):
    """Host-side sharding/layout prep -> per-core input dicts."""
    x2d = np.asarray(x, dtype=np.float32).reshape(M, K)
    xt = _round_f32r(np.ascontiguousarray(x2d.T))          # (K, M)
    cb = _round_f32r(np.asarray(codebook, dtype=np.float32))
    idx2d = np.asarray(indices).reshape(K, N // CDIM).astype(np.int16)
    bias_f = np.asarray(bias, dtype=np.float32)

    in_maps = []
    for c in range(NCORES):
        sub = idx2d[:, NC_CBLK * c : NC_CBLK * (c + 1)]    # (4096, 8)
        # gather position i = cl*128 + p of k-tile t reads block
        # (row 128t+p, col-block c0+cl)
        arr = sub.reshape(KT, 128, NC_CBLK).transpose(0, 2, 1).reshape(KT, 1024)
        # SWDGE index wrap: position i lives at [i % 16, i // 16], and the
        # 16-partition wrap must be replicated across all 8 Q7 core groups.
        wrapped = arr.reshape(KT, 64, 16).transpose(0, 2, 1)  # (KT, 16, 64)
        full = np.tile(
            wrapped.transpose(1, 0, 2).reshape(16, KT * 64), (8, 1)
        )
        bias_t = np.ascontiguousarray(
            np.broadcast_to(
                bias_f[NC_COLS * c : NC_COLS * (c + 1)], (128, NC_COLS)
            )
        )
        in_maps.append({"xt": xt, "cb": cb, "idx": full, "bias": bias_t})
    return in_maps




# ───────────────────────── v2: gather-free (one-hot matmul) ─────────────────
# The SWDGE dma_gather costs ~8 ns of Q7 descriptor-generation per gathered
# block (32768 blocks/core -> ~250 us), which dominates the kernel.  v2
# removes the gather: the one-hot of each block index is built on the Scalar
# engine (is_equal against an iota), and the PE expands W = onehot.T @ CB
# with small matmuls against the SBUF-resident codebook.  One-hot x bf16
# codebook products are exact in PSUM, so W carries bf16-rounded codebook
# values; x flows as float32r.
BF16 = mybir.dt.bfloat16
XB2 = 3      # x-tile bufs
IB2 = 3      # idx-broadcast bufs
OH2 = 2      # one-hot bufs


@functools.lru_cache(maxsize=2)
def build_nc_v2():
    nc = bacc.Bacc("TRN2", target_bir_lowering=False, debug=False)

    xt = nc.dram_tensor("xt", [K, M], F32R, kind="ExternalInput")
    cbt = nc.dram_tensor("cbt", [128, 2 * CDIM], BF16, kind="ExternalInput")
    iot = nc.dram_tensor("iot", [128, 2], mybir.dt.float32, kind="ExternalInput")
    idxb = nc.dram_tensor("idxb", [128, KT * 1024], BF16, kind="ExternalInput")
    bias = nc.dram_tensor("bias", [128, NC_COLS], mybir.dt.float32, kind="ExternalInput")
    out = nc.dram_tensor("out", [M, NC_COLS], mybir.dt.float32, kind="ExternalOutput")

    from contextlib import ExitStack

    with (
        nc.sbuf_tensor("cbs", [128, 2 * CDIM + 2], BF16) as cbs,
        nc.sbuf_tensor("btile", [128, NC_COLS], mybir.dt.float32) as btile,
        ExitStack() as stack,
    ):
        ios = cbs  # iota scalars live in cbs cols 128..129
        xbuf = [
            stack.enter_context(nc.sbuf_tensor(f"xbuf{i}", [128, M], F32R))
            for i in range(XB2)
        ]
        ibuf = [
            stack.enter_context(nc.sbuf_tensor(f"ibuf{i}", [128, 1024], BF16))
            for i in range(IB2)
        ]
        ohb = [
            stack.enter_context(nc.sbuf_tensor(f"oh{i}", [128, 2, 1024], BF16))
            for i in range(OH2)
        ]
        x7buf = stack.enter_context(
            nc.sbuf_tensor("x7buf", [128, KT, 128], F32R)
        )
        wsb = [
            stack.enter_context(nc.sbuf_tensor(f"wsb{t}", [128, NC_COLS], F32R))
            for t in range(KT)
        ]
        obuf = [
            stack.enter_context(
                nc.sbuf_tensor(f"obuf{m}", [128, NC_COLS], mybir.dt.float32)
            )
            for m in range(MT)
        ]
        # 8 PSUM banks: ps[0..6] accumulate out rows m=0..6 over the whole
        # k-loop; ps[7] is the W-build bank during the k-loop and the m=7
        # accumulator in phase B.
        psum = [
            stack.enter_context(
                nc.psum_tensor(f"ps{m}", [128, NC_COLS], mybir.dt.float32)
            )
            for m in range(MT)
        ]
        sxs = [stack.enter_context(nc.semaphore(f"sx{i}")) for i in range(XB2)]
        sx7 = stack.enter_context(nc.semaphore("sx7"))
        sis = [stack.enter_context(nc.semaphore(f"si{i}")) for i in range(IB2)]
        sc = stack.enter_context(nc.semaphore("sc"))    # consts + bias loaded
        sa = stack.enter_context(nc.semaphore("sa"))    # ACT compares done
        sp = stack.enter_context(nc.semaphore("sp"))    # one-hot MM groups done
        sd = stack.enter_context(nc.semaphore("sd"))    # W copies done (DVE)
        sm = stack.enter_context(nc.semaphore("sm"))    # main MM k-tiles done
        smb = stack.enter_context(nc.semaphore("smb"))  # phase-B MMs done
        sv = stack.enter_context(nc.semaphore("sv"))    # bias-adds done
        so = stack.enter_context(nc.semaphore("so"))    # out stores done

        with nc.Block() as block:

            @block.sync
            def _(sync):
                sync.dma_start(cbs[:], cbt[:]).then_inc(sc, 16)
                sync.dma_start(ios[:], iot[:]).then_inc(sc, 16)
                sync.dma_start(btile[:], bias[:]).then_inc(sc, 16)
                # resident copy of every k-tile's m=7 x-slice for phase B
                sync.dma_start(
                    x7buf[:],
                    xt[:, 128 * (MT - 1) :].rearrange("(t p) m -> p t m", p=128),
                ).then_inc(sx7, 16)
                for t in range(KT):
                    if t >= IB2:
                        # idx slot reuse: ACT compares of tile t-IB2 done
                        sync.wait_ge(sa, 2 * (t - IB2 + 1))
                    sync.dma_start(
                        ibuf[t % IB2][:], idxb[:, 1024 * t : 1024 * (t + 1)]
                    ).then_inc(sis[t % IB2], 16)
                    if t >= XB2:
                        # x slot reuse: main MMs of tile t-XB2 done
                        sync.wait_ge(sm, t - XB2 + 1)
                    sync.dma_start(
                        xbuf[t % XB2][:], xt[128 * t : 128 * (t + 1), :]
                    ).then_inc(sxs[t % XB2], 16)
                for m in range(MT):
                    sync.wait_ge(sv, m + 1)
                    sync.dma_start(
                        out[128 * m : 128 * (m + 1), :], obuf[m][:]
                    ).then_inc(so, 16)
                sync.wait_ge(so, 16 * MT)

            @block.tensor
            def _(tensor):
                for t in range(KT):
                    # one-hot expansion of W k-tile t into ps[7]
                    tensor.wait_ge(sa, 2 * (t + 1))
                    if t > 0:
                        # ps[7] free once DVE copied W of tile t-1
                        tensor.wait_ge(sd, t)
                    for cp in range(NC_CBLK):
                        oh = ohb[t % OH2]
                        for h in range(2):
                            ins = tensor.matmul(
                                psum[7][:, CDIM * cp : CDIM * (cp + 1)],
                                oh[:, h, 128 * cp : 128 * (cp + 1)],
                                cbs[:, CDIM * h : CDIM * (h + 1)],
                                start=(h == 0),
                                stop=(h == 1),
                            )
                    ins.then_inc(sp, 1)
                    # main MMs for k-tile t-1 (W already in SBUF)
                    if t > 0:
                        tensor.wait_ge(sxs[(t - 1) % XB2], 16 * ((t - 1) // XB2 + 1))
                        tensor.wait_ge(sd, t)
                        for m in range(MT - 1):
                            ins = tensor.matmul(
                                psum[m][:],
                                xbuf[(t - 1) % XB2][:, 128 * m : 128 * (m + 1)],
                                wsb[t - 1][:],
                                start=(t - 1 == 0),
                                stop=(t - 1 == KT - 1),
                            )
                        ins.then_inc(sm, 1)
                # last k-tile main MMs
                t = KT - 1
                tensor.wait_ge(sxs[t % XB2], 16 * (t // XB2 + 1))
                tensor.wait_ge(sd, KT)
                for m in range(MT - 1):
                    ins = tensor.matmul(
                        psum[m][:],
                        xbuf[t % XB2][:, 128 * m : 128 * (m + 1)],
                        wsb[t][:],
                        start=False,
                        stop=True,
                    )
                ins.then_inc(sm, 1)
                # phase B: m = 7 over all resident W tiles, into ps[7]
                tensor.wait_ge(sx7, 16)
                for t in range(KT):
                    ins = tensor.matmul(
                        psum[7][:],
                        x7buf[:, t, :],
                        wsb[t][:],
                        start=(t == 0),
                        stop=(t == KT - 1),
                        skip_group_check=True,
                    )
                ins.then_inc(smb, 1)

            @block.scalar
            def _(scalar):
                for t in range(KT):
                    scalar.wait_ge(sp, t + 1)
                    scalar.copy(wsb[t][:], psw[t % 3][:]).then_inc(sd, 1)



            @block.vector
            def _(vector):
                vector.wait_ge(sc, 48)
                for t in range(KT):
                    vector.wait_ge(sis[t % IB2], 16 * (t // IB2 + 1))
                    if t >= OH2:
                        # one-hot slot reuse: PE one-hot MMs of t-OH2 done
                        vector.wait_ge(sp, t - OH2 + 1)
                    for h in range(2):
                        vector.tensor_single_scalar(
                            ohb[t % OH2][:, h, :],
                            ibuf[t % IB2][:],
                            ios[:, 128 + h : 129 + h],
                            mybir.AluOpType.is_equal,
                        ).then_inc(sa, 1)
                    if t >= 1:
                        vector.wait_ge(sp, t)
                        vector.tensor_copy(wsb[t - 1][:], psum[7][:]).then_inc(sd, 1)
                vector.wait_ge(sp, KT)
                vector.tensor_copy(wsb[KT - 1][:], psum[7][:]).then_inc(sd, 1)
                vector.wait_ge(sm, KT)
                for m in range(MT - 1):
                    vector.tensor_add(obuf[m][:], psum[m][:], btile[:]).then_inc(sv, 1)
                vector.wait_ge(smb, 1)
                vector.tensor_add(
                    obuf[MT - 1][:], psum[7][:], btile[:]
                ).then_inc(sv, 1)

    nc.compile()
    return nc


def _prep_inputs_v2(x, codebook, bias, indices):
    import ml_dtypes

    x2d = np.asarray(x, dtype=np.float32).reshape(M, K)
    xt = _round_f32r(np.ascontiguousarray(x2d.T))
    cb = np.asarray(codebook, dtype=np.float32)
    iot = np.stack([np.arange(128), np.arange(128) + 128], axis=1)
    cbio = np.ascontiguousarray(
        np.concatenate(
            [cb[:128], cb[128:], iot], axis=1
        ).astype(ml_dtypes.bfloat16)
    )
    idx2d = np.asarray(indices).reshape(K, N // CDIM)
    bias_f = np.asarray(bias, dtype=np.float32)

    in_maps = []
    for c in range(NCORES):
        sub = idx2d[:, NC_CBLK * c : NC_CBLK * (c + 1)]
        arr = sub.reshape(KT, 128, NC_CBLK).transpose(0, 2, 1).reshape(-1)
        idxb = np.ascontiguousarray(
            np.broadcast_to(
                arr.astype(ml_dtypes.bfloat16)[None, :], (128, KT * 1024)
            )
        )
        bias_t = np.ascontiguousarray(
            np.broadcast_to(
                bias_f[NC_COLS * c : NC_COLS * (c + 1)], (128, NC_COLS)
            )
        )
        in_maps.append(
            {"xt": xt, "cbt": cbt, "iot": iot, "idxb": idxb, "bias": bias_t}
        )
    return in_maps


# ───────────────────────── v3: phase-split, bf16, lean DMA ──────────────────
# v2's critical path interleaves W-build (16 small matmuls whose LDWEIGHTS
# dominate) with the main GEMM on every k-tile while 7 of 8 PSUM banks pin
# m-tile accumulators, and it burns HBM on an 8 MB broadcast idx tensor plus
# a redundant 2 MB x re-load (x7buf).  Each 128-partition dma_start also costs
# ~630 ns of descriptor generation on the issuing engine (DIRECT2D), so DMA
# issue itself must be batched.  v3 restructures:
#   - out rows are processed in two m-groups of 4.  Pass A builds each W
#     k-tile (one-hot matmuls, PSUM banks 6-7 double-buffered) and runs the
#     m=0..3 matmuls; pass B reuses the SBUF-resident x and W tiles for
#     m=4..7 (banks 4-5 plus the freed 6-7) with zero DMA.
#   - x is bf16 and fully SBUF-resident (8 MB), loaded as 8 1-MB group DMAs
#     (128 descriptors of 8 KB each).  Within group g, partition p holds the
#     four k-rows 512g+4p+a (a=0..3); the W-build emits the same permuted
#     k-order, so the PE contraction is unchanged.
#   - idx (bf16, replicated across partitions for the one-hot compare) is
#     streamed in 4-k-tile groups through a 3-slot ring.
#   - the PSUM->SBUF W cast runs on the otherwise-idle Scalar engine.
XB3 = 4      # unused (x is resident); kept for the legacy v2 path
IB3 = 3      # idx group bufs
OH3 = 2      # one-hot bufs
KG = 4       # k-tiles per DMA group
NG = KT // KG  # 8 groups


@functools.lru_cache(maxsize=4)
def build_nc_v3(warmup=0):
    nc = bacc.Bacc("TRN2", target_bir_lowering=False, debug=False)

    xtg = nc.dram_tensor("xtg", [128, KT * 1024], BF16, kind="ExternalInput")
    cbio = nc.dram_tensor("cbio", [128, 2 * CDIM + 2], BF16, kind="ExternalInput")
    idxb = nc.dram_tensor("idxb", [128, KT * 1024], BF16, kind="ExternalInput")
    bias = nc.dram_tensor("bias", [128, NC_COLS], mybir.dt.float32, kind="ExternalInput")
    out = nc.dram_tensor("out", [M, NC_COLS], mybir.dt.float32, kind="ExternalOutput")

    from contextlib import ExitStack

    with (
        nc.sbuf_tensor("cbs", [128, 2 * CDIM + 2], BF16) as cbs,
        nc.sbuf_tensor("btile", [128, NC_COLS], mybir.dt.float32) as btile,
        ExitStack() as stack,
    ):
        ios = cbs  # iota scalars live in cbs cols 128..129
        xg = [
            stack.enter_context(nc.sbuf_tensor(f"xg{g}", [128, KG, 1024], BF16))
            for g in range(NG)
        ]
        ibuf = [
            stack.enter_context(nc.sbuf_tensor(f"ibuf{i}", [128, KG * 1024], BF16))
            for i in range(IB3)
        ]
        ohb = [
            stack.enter_context(nc.sbuf_tensor(f"oh{i}", [128, 2, 1024], BF16))
            for i in range(OH3)
        ]
        wsb = [
            stack.enter_context(nc.sbuf_tensor(f"wsb{t}", [128, NC_COLS], BF16))
            for t in range(KT)
        ]
        obuf = [
            stack.enter_context(
                nc.sbuf_tensor(f"obuf{m}", [128, NC_COLS], mybir.dt.float32)
            )
            for m in range(MT)
        ]
        # 8 PSUM banks: 0-3 pass-A accumulators (m=0..3), 4-5 pass-B (m=4,5),
        # 6-7 W-build double-buffer in pass A then pass-B m=6,7.
        psm = [
            stack.enter_context(
                nc.psum_tensor(f"ps{m}", [128, NC_COLS], mybir.dt.float32)
            )
            for m in range(5)
        ]
        psw = [
            stack.enter_context(
                nc.psum_tensor(f"pw{i}", [128, NC_COLS], mybir.dt.float32)
            )
            for i in range(3)
        ]
        sidx = [stack.enter_context(nc.semaphore(f"si{i}")) for i in range(IB3)]
        sxg = [stack.enter_context(nc.semaphore(f"sxg{g}")) for g in range(NG)]
        sif = stack.enter_context(nc.semaphore("sif"))  # first idx k-tile
        sc = stack.enter_context(nc.semaphore("sc"))    # cb/iota consts loaded
        sb = stack.enter_context(nc.semaphore("sb"))    # bias loaded
        sa = stack.enter_context(nc.semaphore("sa"))    # is_eq done
        sp = stack.enter_context(nc.semaphore("sp"))    # build MM groups done
        sd = stack.enter_context(nc.semaphore("sd"))    # W casts done
        sma = stack.enter_context(nc.semaphore("sma"))  # pass-A main k-tiles
        smb = stack.enter_context(nc.semaphore("smb"))  # pass-B main k-tiles
        sv = stack.enter_context(nc.semaphore("sv"))    # bias adds
        so = stack.enter_context(nc.semaphore("so"))    # out stores

        with nc.Block() as block:

            @block.sync
            def _(sync):
                # first k-tile's idx chunk leads; everything else follows
                sync.dma_start(ibuf[0][:, 0:1024], idxb[:, 0:1024]).then_inc(sif, 16)
                sync.dma_start(cbs[:], cbio[:]).then_inc(sc, 16)
                for g in range(NG):
                    if g >= IB3:
                        sync.wait_ge(sa, 8 * (g - IB3 + 1))
                    if g == 0:
                        sync.dma_start(
                            ibuf[0][:, 1024:4096], idxb[:, 1024:4096]
                        ).then_inc(sidx[0], 16)
                    else:
                        sync.dma_start(
                            ibuf[g % IB3][:], idxb[:, 4096 * g : 4096 * (g + 1)]
                        ).then_inc(sidx[g % IB3], 16)
                    sync.dma_start(
                        xg[g].ap().rearrange("p a m -> p (a m)"),
                        xtg[:, 4096 * g : 4096 * (g + 1)],
                    ).then_inc(sxg[g], 16)
                sync.dma_start(btile[:], bias[:]).then_inc(sb, 16)
                for m in range(MT):
                    sync.wait_ge(sv, m + 1)
                    sync.dma_start(
                        out[128 * m : 128 * (m + 1), :], obuf[m][:]
                    ).then_inc(so, 16)
                sync.wait_ge(so, 16 * MT)

            @block.tensor
            def _(tensor):
                # p-state warmup: waitless junk matmuls from engine start so
                # the PE is at full clock when the first real build arrives.
                # Reads uninitialized SBUF (results discarded; psum banks are
                # reset by the first start=True group), so CoreSim runs with
                # warmup=0.
                for w in range(warmup):
                    tensor.matmul(
                        psw[0][:],
                        ohb[0][:, 1, 0:128],
                        ohb[0][:, 0, 0:512],
                        start=True,
                        stop=True,
                        skip_group_check=True,
                    )
                for t in range(KT):
                    # W-build for k-tile t into psw[t%3].  Tile 0 gates its
                    # h=0 matmuls on the first is_eq only (h-major order) so
                    # the PE starts ~0.4us earlier.
                    if t >= 3:
                        tensor.wait_ge(sd, t - 2)  # cast of t-3 freed the bank
                    oh = ohb[t % OH3]
                    tensor.wait_ge(sa, 2 * (t + 1))
                    for cp in range(NC_CBLK):
                        for h in range(2):
                            ins = tensor.matmul(
                                psw[t % 3][:, CDIM * cp : CDIM * (cp + 1)],
                                oh[:, h, 128 * cp : 128 * (cp + 1)],
                                cbs[:, CDIM * h : CDIM * (h + 1)],
                                start=(h == 0),
                                stop=(h == 1),
                            )
                    ins.then_inc(sp, 1)
                    # pass-A mains for k-tile t-1
                    if t >= 1:
                        tm = t - 1
                        tensor.wait_ge(sd, tm + 1)
                        if tm % KG == 0:
                            tensor.wait_ge(sxg[tm // KG], 16)
                        for m in range(4):
                            ins = tensor.matmul(
                                psm[m][:],
                                xg[tm // KG][:, tm % KG, 128 * m : 128 * (m + 1)],
                                wsb[tm][:],
                                start=(tm == 0),
                                stop=(tm == KT - 1),
                            )
                tm = KT - 1
                tensor.wait_ge(sd, KT)
                for m in range(4):
                    ins = tensor.matmul(
                        psm[m][:],
                        xg[tm // KG][:, tm % KG, 128 * m : 128 * (m + 1)],
                        wsb[tm][:],
                        start=False,
                        stop=True,
                    )
                ins.then_inc(sma, 1)
                # pass B: m = 4..7 over resident x and W tiles, zero DMA.
                # m-major so each m's bias-add/store overlaps the next m's
                # matmuls instead of all serializing at the end.
                for m in range(4):
                    dst = psw[m] if m < 3 else psm[4]
                    for t in range(KT):
                        ins = tensor.matmul(
                            dst[:],
                            xg[t // KG][:, t % KG, 512 + 128 * m : 512 + 128 * (m + 1)],
                            wsb[t][:],
                            start=(t == 0),
                            stop=(t == KT - 1),
                            skip_group_check=True,
                        )
                    ins.then_inc(smb, 1)

            @block.scalar
            def _(scalar):
                for t in range(KT):
                    scalar.wait_ge(sp, t + 1)
                    scalar.copy(wsb[t][:], psw[t % 3][:]).then_inc(sd, 1)



            @block.vector
            def _(vector):
                vector.wait_ge(sc, 16)
                for t in range(KT):
                    g = t // KG
                    if t == 0:
                        vector.wait_ge(sif, 16)
                    else:
                        vector.wait_ge(sidx[g % IB3], 16 * (g // IB3 + 1))
                    if t >= OH3:
                        vector.wait_ge(sp, t - OH3 + 1)
                    for h in range(2):
                        vector.tensor_single_scalar(
                            ohb[t % OH3][:, h, :],
                            ibuf[g % IB3][:, 1024 * (t % KG) : 1024 * (t % KG + 1)],
                            ios[:, 128 + h : 129 + h],
                            mybir.AluOpType.is_equal,
                        ).then_inc(sa, 1)
                vector.wait_ge(sb, 16)
                vector.wait_ge(sma, 1)
                for m in range(4):
                    vector.tensor_add(obuf[m][:], psm[m][:], btile[:]).then_inc(sv, 1)
                for m in range(4):
                    vector.wait_ge(smb, m + 1)
                    src = psw[m] if m < 3 else psm[4]
                    vector.tensor_add(obuf[4 + m][:], src[:], btile[:]).then_inc(sv, 1)

    nc.compile()
    return nc


def _prep_inputs_v3(x, codebook, bias, indices):
    import ml_dtypes

    x2d = np.asarray(x, dtype=np.float32).reshape(M, K)
    xt = np.ascontiguousarray(x2d.T).astype(ml_dtypes.bfloat16)  # (K, M)
    # group-permuted resident layout: partition p of group g holds k-rows
    # 512g + 4p + a (a = 0..3), so each DMA descriptor covers 4 contiguous
    # k-rows (8 KB).
    xtg = np.ascontiguousarray(
        xt.reshape(NG, 128, KG, M).transpose(1, 0, 2, 3).reshape(128, KT * 1024)
    )
    cb = np.asarray(codebook, dtype=np.float32)
    iot = np.stack([np.arange(128), np.arange(128) + 128], axis=1)
    cbio = np.ascontiguousarray(
        np.concatenate(
            [cb[:128], cb[128:], iot], axis=1
        ).astype(ml_dtypes.bfloat16)
    )
    idx2d = np.asarray(indices).reshape(K, N // CDIM)
    bias_f = np.asarray(bias, dtype=np.float32)

    in_maps = []
    for c in range(NCORES):
        sub = idx2d[:, NC_CBLK * c : NC_CBLK * (c + 1)]    # (4096, 8)
        # k-tile t = 4g + a covers k-rows 512g + 4j + a at psum partition j:
        # is_eq column (g, a, cp, j) must hold idx2d[512g + 4j + a, 8c + cp].
        arr = (
            sub.reshape(NG, 128, KG, NC_CBLK)
            .transpose(0, 2, 3, 1)                         # (g, a, cp, j)
            .reshape(-1)
        )
        idxb = np.ascontiguousarray(
            np.broadcast_to(
                arr.astype(ml_dtypes.bfloat16)[None, :], (128, KT * 1024)
            )
        )
        bias_t = np.ascontiguousarray(
            np.broadcast_to(
                bias_f[NC_COLS * c : NC_COLS * (c + 1)], (128, NC_COLS)
            )
        )
        in_maps.append(
            {"xtg": xtg, "cbio": cbio, "idxb": idxb, "bias": bias_t}
        )
    return in_maps


def kernel(x, codebook, continuous_weight, bias, indices):
    # continuous_weight cancels in the forward pass (see module docstring).
    del continuous_weight
    nc = build_nc_v3()
    in_maps = _prep_inputs_v3(x, codebook, bias, indices)
    res = run_bass_kernel_spmd(nc, in_maps, core_ids=list(range(NCORES)))
    cols = [res.results[c]["out"] for c in range(NCORES)]
    full = np.concatenate(cols, axis=1)
    return full.reshape(2, 512, N).astype(np.float32)

